# revision 68
# baseline (speedup 1.0000x reference)
"""Trainium2 Bass kernel for nn_Balancer_10660108829428.

Computes (total, fg_loss, bg_loss) for a fg/bg-weighted loss balancer:
  fg_mask[b,h,w] = any of 48 boxes covers pixel (h,w)
  fg_loss = 13 * sum(loss * fg) / (B*H*W)
  bg_loss = sum(loss * ~fg) / (B*H*W)
  total   = fg_loss + bg_loss

Strategy: data-parallel over B across 8 NeuronCores (8 batches each).
Per core each batch's mask is rasterized as a rank-48 matmul
(row_inT @ col_in) on the TensorEngine in bf16; a single fused DVE
scalar_tensor_tensor op computes (counts > 0) * loss with a free
per-partition row-sum accumulator (S_fg); ScalarE accumulates S_all
via activation(Copy, accum_out=...); the Pool engine builds the
per-box row/col interval masks. Each core emits a [1,4] partial
(S_fg_full, S_all_full, S_fg_tail, S_all_tail); the host sums them.

Box membership avoids floor/ceil entirely: for integer h,
  h >= floor(v1)  <=>  h > v1 - 1      and      h < ceil(v2)  <=>  h < v2.
"""

import numpy as np

import concourse.bacc as bacc
import concourse.mybir as mybir
import concourse.tile as tile
from concourse.bass_utils import run_bass_kernel_spmd

B, H, W, N = 64, 376, 1248, 48
N_CORES = 8
BPC = B // N_CORES          # batches per core
PAIRS = BPC // 2            # batch pairs per core (masks built 2 batches at a time)
FG_WEIGHT = 13.0
H_CHUNKS = [(0, 128), (128, 128), (256, H - 256)]  # (h0, hsz)
F32 = mybir.dt.float32
BF16 = mybir.dt.bfloat16

_NC_CACHE = None


def _build_nc(
    lt_bufs=5, sc_bufs=2, cnt_bufs=2, mask_bufs=4, box_dma_gpsimd=False,
    masks_on_pool="rest", prebuild_masks=True, skip=(), split_dma=1,
    tail_split=True, fuse_hc01=False, sall_last=False, iota_pool=False,
):
    # Bacc (not bass.Bass): its finalize() runs the TRN2 wait-legalization
    # passes (move_matmul_waits_to_ldweights / generate_event_semaphores) —
    # the ISA allows only one semaphore wait per instruction.
    nc = bacc.Bacc("TRN2")
    loss_d = nc.dram_tensor("loss", [BPC, H, W], F32, kind="ExternalInput")
    boxes_d = nc.dram_tensor("boxes", [BPC, N, 4], F32, kind="ExternalInput")
    # (S_fg_full, S_all_full, S_fg_tail, S_all_tail) — host sums the pairs
    out_d = nc.dram_tensor("out", [1, 4], F32, kind="ExternalOutput")

    AX = mybir.AxisListType
    OP = mybir.AluOpType
    AF = mybir.ActivationFunctionType

    with tile.TileContext(nc) as tc:
        with (
            tc.tile_pool(name="singles", bufs=1) as singles,
            tc.tile_pool(name="masks", bufs=mask_bufs) as masks,
            tc.tile_pool(name="ltiles", bufs=lt_bufs) as ltiles,
            tc.tile_pool(name="scratch", bufs=sc_bufs) as scratch,
            tc.tile_pool(name="cpsum", bufs=cnt_bufs, space="PSUM") as cpsum,
            tc.tile_pool(name="opsum", bufs=1, space="PSUM") as opsum,
        ):
            # --- constants ---
            iota_i = singles.tile([128, W], mybir.dt.int32)
            nc.gpsimd.iota(iota_i, pattern=[[1, W]], base=0, channel_multiplier=0)
            iota_f = singles.tile([128, W], F32)
            (nc.gpsimd if iota_pool else nc.vector).tensor_copy(iota_f, iota_i)
            ones = singles.tile([128, 1], F32)
            nc.vector.memset(ones, 1.0)
            # accum slots are written (not accumulated) by accum_out, so no
            # zero-init is needed.  Slots for the 120-row h-chunk (hc==2)
            # only cover partitions 0..119; they are grouped at the tail so
            # the final reduction can treat them separately.
            acc_fg = singles.tile([128, 26], F32)   # slot = hc*8 + b (b = batch 0..7)
            # slot = hc*4 + p; slots 12.. = extra tail-split segments
            acc_all = singles.tile([128, 15], F32)
            TS = 2 if tail_split == 2 else 1  # w-segments per batch in the tail
            WS = W // TS

            # batch-in-pair q lives at partition base 64*q (matmul requires
            # operand base partitions of 0/32/64); partitions 48..63 are
            # zeroed padding.
            NP = 64 + N  # 112 partitions spanned by the two batches

            # All boxes in two DMAs, already in the (q*64+n) partition layout
            # used by the mask builds: bx_all[q*64+n, 4*p+c] = boxes[2p+q, n, c].
            bx_all = singles.tile([128, 4 * PAIRS], F32)
            nc.vector.memset(bx_all, 0.0)
            src_q = boxes_d.rearrange("(p two) n c -> two p n c", two=2)
            for q in range(2):
                nc.sync.dma_start(
                    out=bx_all[64 * q : 64 * q + N].rearrange(
                        "n (p c) -> n p c", p=PAIRS
                    ),
                    in_=src_q[q].rearrange("p n c -> n p c"),
                )
            # (u1-1, v1-1) per box-instance, all pairs in one op
            bm1_all = singles.tile([128, 2 * PAIRS], F32)
            nc.vector.tensor_scalar(
                bm1_all[:NP].rearrange("n (p c) -> n p c", p=PAIRS),
                bx_all[:NP].rearrange("n (p c) -> n p c", p=PAIRS)[:, :, 0:2],
                1.0,
                None,
                OP.subtract,
            )

            def build_masks(p):
                on_pool = masks_on_pool and not (masks_on_pool == "rest" and p == 0)
                eng = nc.gpsimd if on_pool else nc.vector
                bx = bx_all[:, 4 * p : 4 * (p + 1)]
                bm1 = bm1_all[:, 2 * p : 2 * (p + 1)]
                # rows: (h > v1-1) & (h < v2)  as bf16
                rowa = masks.tile([128, H], BF16, tag="rowa")
                eng.tensor_scalar(
                    rowa[:NP], iota_f[:NP, :H], bm1[:NP, 1:2], None, OP.is_gt
                )
                rowb = masks.tile([128, H], BF16, tag="rowb")
                eng.tensor_scalar(
                    rowb[:NP], iota_f[:NP, :H], bx[:NP, 3:4], None, OP.is_lt
                )
                rowm = masks.tile([128, H], BF16, tag="rowm")
                eng.tensor_tensor(rowm[:NP], rowa[:NP], rowb[:NP], OP.mult)
                # cols: (w > u1-1) & (w < u2)  as bf16
                cola = masks.tile([128, W], BF16, tag="cola")
                eng.tensor_scalar(
                    cola[:NP], iota_f[:NP, :], bm1[:NP, 0:1], None, OP.is_gt
                )
                colb = masks.tile([128, W], BF16, tag="colb")
                eng.tensor_scalar(
                    colb[:NP], iota_f[:NP, :], bx[:NP, 2:3], None, OP.is_lt
                )
                colm = masks.tile([128, W], BF16, tag="colm")
                eng.tensor_tensor(colm[:NP], cola[:NP], colb[:NP], OP.mult)
                return rowm, colm

            prebuilt = (
                [build_masks(p) for p in range(PAIRS)] if prebuild_masks else None
            )
            for p in range(PAIRS):
                rowm, colm = prebuilt[p] if prebuilt else build_masks(p)
                for hc, (h0, hsz) in enumerate(H_CHUNKS):
                    is_tail = tail_split and p == PAIRS - 1 and hc == 2
                    # ~1.2MB of loss: both batches of the pair, same h-chunk
                    lt = ltiles.tile([128, 2 * W], F32, tag="lt")
                    lt_full = lt[:hsz]
                    if is_tail:
                        # per-batch (and optionally per-w-half) DMAs: shrinks
                        # the trailing DMA->compute chain at the kernel end
                        for q in range(2):
                            for wh in range(TS):
                                nc.sync.dma_start(
                                    out=lt[
                                        :hsz, q * W + wh * WS : q * W + (wh + 1) * WS
                                    ],
                                    in_=loss_d[
                                        2 * p + q, h0 : h0 + hsz,
                                        wh * WS : (wh + 1) * WS,
                                    ],
                                )
                    else:
                        nc.sync.dma_start(
                            out=lt[:hsz].rearrange("h (b w) -> h b w", b=2),
                            in_=loss_d[
                                2 * p : 2 * p + 2, h0 : h0 + hsz, :
                            ].rearrange("b h w -> h b w"),
                        )
                    # S_all partial on ScalarE (free accumulator)
                    def emit_sall():
                        if is_tail:
                            for q in range(2):
                                for wh in range(TS):
                                    sa = scratch.tile([128, 2 * W], F32, tag="sa")
                                    slot = 11 + q + 2 * wh  # 11 = normal slot
                                    nc.scalar.activation(
                                        out=sa[:hsz, :WS],
                                        in_=lt_full[
                                            :, q * W + wh * WS : q * W + (wh + 1) * WS
                                        ],
                                        func=AF.Copy,
                                        accum_out=acc_all[:hsz, slot : slot + 1],
                                    )
                        else:
                            sa = scratch.tile([128, 2 * W], F32, tag="sa")
                            nc.scalar.activation(
                                out=sa[:hsz],
                                in_=lt_full,
                                func=AF.Copy,
                                accum_out=acc_all[:hsz, hc * 4 + p : hc * 4 + p + 1],
                            )

                    if "sall" not in skip and not sall_last:
                        emit_sall()
                    for q in range(2):
                        if "fg" in skip:
                            continue
                        b_idx = 2 * p + q
                        cnt = cpsum.tile([128, W], F32, tag="cnt")
                        for w0 in range(0, W, 512):
                            wsz = min(512, W - w0)
                            nc.tensor.matmul(
                                cnt[:hsz, w0 : w0 + wsz],
                                lhsT=rowm[64 * q : 64 * q + N, h0 : h0 + hsz],
                                rhs=colm[64 * q : 64 * q + N, w0 : w0 + wsz],
                                start=True,
                                stop=True,
                            )
                        if "stt" in skip:
                            continue
                        # fused: (counts > 0) * loss, with row-sum accumulator
                        sf = scratch.tile([128, W], F32, tag="sf")
                        for wh in range(TS if is_tail else 1):
                            wn = W if not is_tail else WS
                            slot = (
                                hc * 8 + b_idx if not is_tail else 22 + q + 2 * wh
                            )
                            nc.vector.scalar_tensor_tensor(
                                sf[:hsz, wh * wn : (wh + 1) * wn],
                                cnt[:hsz, wh * wn : (wh + 1) * wn],
                                0.0,
                                lt_full[:, q * W + wh * wn : q * W + (wh + 1) * wn],
                                op0=OP.is_gt,
                                op1=OP.mult,
                                accum_out=acc_fg[:hsz, slot : slot + 1],
                            )
                    if "sall" not in skip and sall_last:
                        emit_sall()

            # --- final on-core reduction ---
            # cols 0,1: full-partition slots (hc 0,1) reduced over all 128
            # partitions; cols 2,3: hc==2 slots over partitions 0..119 with
            # rows 96..127 zeroed first (engine partition base must be
            # 0/32/64/96; the tail reduces overwrite rows 0..119) so ONE
            # K=128 ones-matmul folds everything into a [1,4] psum.
            red = singles.tile([128, 4], F32)
            nc.vector.memset(red[96:, 2:4] if not skip else red[:, :], 0.0)
            if not skip:
                nc.vector.tensor_reduce(
                    red[:, 0:1], acc_fg[:, 0:16], axis=AX.X, op=OP.add
                )
                nc.vector.tensor_reduce(
                    red[:, 1:2], acc_all[:, 0:8], axis=AX.X, op=OP.add
                )
                fg_hi = 26 if TS == 2 else 24
                nc.vector.tensor_reduce(
                    red[:120, 2:3], acc_fg[:120, 16:fg_hi], axis=AX.X, op=OP.add
                )
                all_hi = (11 + 2 * TS) if tail_split else 12
                nc.vector.tensor_reduce(
                    red[:120, 3:4], acc_all[:120, 8:all_hi], axis=AX.X, op=OP.add
                )
            out_ps = opsum.tile([1, 4], F32)
            nc.tensor.matmul(out_ps, lhsT=ones, rhs=red, start=True, stop=True)
            out_sb = singles.tile([1, 4], F32)
            nc.vector.tensor_copy(out_sb, out_ps)
            nc.sync.dma_start(out=out_d[:, :], in_=out_sb)

    nc.finalize()
    return nc


def get_nc():
    global _NC_CACHE
    if _NC_CACHE is None:
        _NC_CACHE = _build_nc()
    return _NC_CACHE


def run_cores(loss, gt_boxes2d, trace=False, **kw):
    loss = np.ascontiguousarray(loss, dtype=np.float32)
    boxes = np.ascontiguousarray(gt_boxes2d, dtype=np.float32)
    in_maps = [
        {
            "loss": np.ascontiguousarray(loss[c * BPC : (c + 1) * BPC]),
            "boxes": np.ascontiguousarray(boxes[c * BPC : (c + 1) * BPC]),
        }
        for c in range(N_CORES)
    ]
    return run_bass_kernel_spmd(
        get_nc(), in_maps, core_ids=list(range(N_CORES)), trace=trace, **kw
    )


def kernel(loss, gt_boxes2d):
    res = run_cores(loss, gt_boxes2d)
    s_fg = 0.0
    s_all = 0.0
    for r in res.results:
        o = r["out"][0]
        s_fg += float(o[0]) + float(o[2])
        s_all += float(o[1]) + float(o[3])
    n_pix = float(B * H * W)
    fg_loss = FG_WEIGHT * s_fg / n_pix
    bg_loss = (s_all - s_fg) / n_pix
    total = fg_loss + bg_loss
    return (
        np.array(total, dtype=np.float32),
        np.array(fg_loss, dtype=np.float32),
        np.array(bg_loss, dtype=np.float32),
    )
